# revision 3
# baseline (speedup 1.0000x reference)
"""CommNet actor kernel v2: g-space aggregation with embedded degree.

Key idea: msg enters the MLP only through W2m, and aggregation commutes with
linear projection: W2m.T @ (adj@h).T = Q @ (adj @ (h@P.T)).T for any
factorization W2m.T = Q P. Using the rank-127 SVD of W2m (drop sigma_128,
~3e-4 relative error), P is [127,128], so the aggregated feature vector
g = [P h, 1] fits in 128 stationary columns WITH a ones column - the same
fp8 DoubleRow stream that aggregates messages also produces deg in output
row 127. The baseline's second full adjacency stream for deg (~14.5us of
PE time) disappears.

Layouts: h is computed feature-major (hT_all = tanh(W1.T obsT + b1)) which
also serves the MLP's exact-h path directly (E2 eliminated). A G-stage
(64 matmuls, stationary = hT chunk, moving = U127) produces agent-major
g chunks for the aggregation stationary. Inputs are rotated per-core so
own rows sit at agent positions 0..1023 (SPMD program identical).

Per-core PE stream (1 cyc/out-col real rate):
  agg 32768 + E1' 8192 + G 8128 + MLP ~4096 cols  ~= 22us vs baseline ~36.
"""

import numpy as np
import ml_dtypes
from contextlib import ExitStack

import concourse.tile as tile
from concourse import bacc, mybir
from concourse.bass import ts

N_AGENTS, OBS_DIM, HID, ACT_DIM = 8192, 64, 128, 16
CORES = 8
ROWS = N_AGENTS // CORES          # 1024 rows per core
JCH = N_AGENTS // 128             # 64 contraction chunks
GRP = 8                           # j-chunks per adjacency DMA slab
GDIM = HID - 1                    # 127 projected features (+1 ones)

F32 = mybir.dt.float32
F32R = mybir.dt.float32r
BF16 = mybir.dt.bfloat16
FP8 = mybir.dt.float8e4
BF16_NP = ml_dtypes.bfloat16
FP8_NP = ml_dtypes.float8_e4m3
FP8_ONE = 0x38  # bit pattern of 1.0 in e4m3

Tanh = mybir.ActivationFunctionType.Tanh
Identity = mybir.ActivationFunctionType.Identity


def _build_nc(reps=1):
    nc = bacc.Bacc("TRN2", target_bir_lowering=False, debug=False,
                   num_devices=CORES)

    adjT = nc.dram_tensor("adjT", [128, JCH, ROWS], FP8, kind="ExternalInput")
    obsT = nc.dram_tensor("obsT", [OBS_DIM, N_AGENTS], BF16,
                          kind="ExternalInput")
    w1 = nc.dram_tensor("w1", [OBS_DIM, HID], BF16, kind="ExternalInput")
    b1 = nc.dram_tensor("b1", [HID, 1], F32, kind="ExternalInput")
    pt = nc.dram_tensor("pt", [HID, GDIM], BF16, kind="ExternalInput")
    w2h = nc.dram_tensor("w2h", [HID, HID], BF16, kind="ExternalInput")
    # qt row 0 is zero: it multiplies the deg row of the aggregate
    qt = nc.dram_tensor("qt", [HID, HID], F32R, kind="ExternalInput")
    b2 = nc.dram_tensor("b2", [HID, 1], F32, kind="ExternalInput")
    w3 = nc.dram_tensor("w3", [HID, ACT_DIM], F32R, kind="ExternalInput")
    b3 = nc.dram_tensor("b3", [ACT_DIM, 1], F32, kind="ExternalInput")
    logitsT = nc.dram_tensor("logitsT", [ACT_DIM, ROWS], F32,
                             kind="ExternalOutput")

    DR = mybir.MatmulPerfMode.DoubleRow
    NR = ROWS // 512        # 2 output ranges per core
    NSLAB = JCH // GRP      # 8 adjacency slabs
    NW = 16                 # agent windows of 512 (4 chunks each)
    with tile.TileContext(nc) as tc, ExitStack() as ctx:
        consts = ctx.enter_context(tc.tile_pool(name="consts", bufs=1))
        stage = ctx.enter_context(tc.tile_pool(name="stage", bufs=1))
        stage2 = ctx.enter_context(tc.tile_pool(name="stage2", bufs=2))
        adjp = ctx.enter_context(tc.tile_pool(name="adjp", bufs=NSLAB))

        w1_sb = consts.tile([OBS_DIM, HID], BF16, tag="w1")
        nc.sync.dma_start(w1_sb[:], w1[:])
        b1_sb = consts.tile([HID, 1], F32, tag="b1")
        nc.sync.dma_start(b1_sb[:], b1[:])
        pt_sb = consts.tile([HID, GDIM], BF16, tag="pt")
        nc.sync.dma_start(pt_sb[:], pt[:])
        # obsT split into 4 tiles so E1' can start early
        OCH = 4
        ow = N_AGENTS // OCH
        obsT_sbs = []
        early_slabs = []
        for oc in range(OCH):
            t = consts.tile([OBS_DIM, ow], BF16, tag=f"obsT{oc}",
                            name=f"obsT{oc}")
            nc.sync.dma_start(t[:], obsT[:, oc * ow : (oc + 1) * ow])
            obsT_sbs.append(t)
            if oc < 2:
                es = adjp.tile([128, GRP, ROWS], FP8, tag="adjT",
                               name=f"adjT_0_{oc}")
                nc.sync.dma_start(es[:], adjT[:, oc * GRP : (oc + 1) * GRP, :])
                early_slabs.append(es)
        w2h_sb = consts.tile([HID, HID], BF16, tag="w2h")
        nc.sync.dma_start(w2h_sb[:], w2h[:])
        qt_sb = consts.tile([HID, HID], F32R, tag="qt")
        nc.sync.dma_start(qt_sb[:], qt[:])
        b2_sb = consts.tile([HID, 1], F32, tag="b2")
        nc.sync.dma_start(b2_sb[:], b2[:])
        w3_sb = consts.tile([HID, ACT_DIM], F32R, tag="w3")
        nc.sync.dma_start(w3_sb[:], w3[:])
        b3_sb = consts.tile([ACT_DIM, 1], F32, tag="b3")
        nc.sync.dma_start(b3_sb[:], b3[:])
        ones_bc = consts.tile([1, 128], F32R, tag="ones_bc")
        nc.vector.memset(ones_bc[:].bitcast(mybir.dt.uint32), 0x3F800000)

        for rep in range(reps):
            hT = stage2.tile([HID, N_AGENTS], BF16, tag="hT",
                             name=f"hT_{rep}")
            g_sb = stage2.tile([128, JCH, HID], FP8, tag="g_sb",
                               name=f"g_sb_{rep}")
            msgT = stage2.tile([HID, ROWS], F32R, tag="msgT",
                               name=f"msgT_{rep}")
            hidT = stage2.tile([HID, ROWS], F32R, tag="hidT",
                               name=f"hidT_{rep}")
            logT = stage2.tile([ACT_DIM, ROWS], F32, tag="logT",
                               name=f"logT_{rep}")

            # ones feature (column 0 of every chunk) -> deg in psum row 0
            nc.vector.memset(
                g_sb[:, :, 0:1].bitcast(mybir.dt.uint8), FP8_ONE)

            with ExitStack() as rctx:
                pp_agg = rctx.enter_context(
                    tc.tile_pool(name=f"pp_agg_{rep}", bufs=1, space="PSUM"))
                msgps = [pp_agg.tile([128, 512], F32, tag=f"msgps{r}",
                                     name=f"msgps_{rep}_{r}")
                         for r in range(NR)]
                enc_ctx = ExitStack()
                pp_enc = enc_ctx.enter_context(
                    tc.tile_pool(name=f"pp_enc_{rep}", bufs=2, space="PSUM"))

                def enc_mm(q):
                    # E1': hT[:, win] = tanh(W1.T @ obsT[:, win] + b1)
                    osb = obsT_sbs[q * 512 // ow]
                    ocol = (q * 512) % ow
                    ps = pp_enc.tile([HID, 512], F32, tag="e1",
                                     name=f"e1_{rep}_{q}")
                    nc.tensor.matmul(ps[:], w1_sb[:],
                                     osb[:, ocol : ocol + 512],
                                     start=True, stop=True)
                    nc.scalar.activation(hT[:, ts(q, 512)], ps[:], Tanh,
                                         bias=b1_sb[:, 0:1])

                def enc_g(q):
                    # G: agent-major projected features, 4 chunks per bank
                    gp = pp_enc.tile([128, 4, GDIM], F32, tag="g",
                                     name=f"g_{rep}_{q}")
                    for k in range(4):
                        c = 4 * q + k
                        nc.tensor.matmul(gp[:, k, :],
                                         hT[:, c * 128 : (c + 1) * 128],
                                         pt_sb[:], start=True, stop=True)
                    nc.vector.tensor_copy(
                        g_sb[:, 4 * q : 4 * q + 4, 1:HID], gp[:])

                def agg_pairs(r, g, slab):
                    for jj2 in range(GRP // 2):
                        j = g * GRP + 2 * jj2
                        first = (g == 0 and jj2 == 0)
                        last = (g == NSLAB - 1 and jj2 == GRP // 2 - 1)
                        nc.tensor.matmul(msgps[r][:],
                                         g_sb[:, j : j + 2, :],
                                         slab[:, 2 * jj2 : 2 * jj2 + 2,
                                              ts(r, 512)],
                                         start=first, stop=last,
                                         perf_mode=DR)

                def epilogue(r):
                    # deg is row 127 of the aggregate
                    dmax = stage.tile([1, 512], F32R, tag="dmax",
                                      name=f"dmax_{rep}_{r}")
                    nc.vector.tensor_scalar_max(
                        dmax[:], msgps[r][0:1, :], 1.0)
                    bc = pp_mlp.tile([128, 512], F32, tag="bc",
                                     name=f"bc_{rep}_{r}")
                    nc.tensor.matmul(bc[:], ones_bc[:], dmax[:],
                                     start=True, stop=True)
                    recip = stage.tile([128, 512], F32, tag="recip",
                                       name=f"recip_{rep}_{r}")
                    nc.vector.reciprocal(recip[:], bc[:])
                    nc.vector.tensor_tensor(msgT[:, ts(r, 512)],
                                            msgps[r][:], recip[:],
                                            mybir.AluOpType.mult)
                    pw = pp_mlp.tile([128, 512], F32, tag="w2p", bufs=2,
                                     name=f"w2p_{rep}_{r}")
                    nc.tensor.matmul(pw[:], w2h_sb[:], hT[:, ts(r, 512)],
                                     start=True, stop=False)
                    nc.tensor.matmul(pw[:], qt_sb[:],
                                     msgT[:, ts(r, 512)],
                                     start=False, stop=True)
                    nc.scalar.activation(hidT[:, ts(r, 512)], pw[:], Tanh,
                                         bias=b2_sb[:, 0:1])
                    pl = pp_mlp.tile([ACT_DIM, 512], F32, tag="w3p",
                                     name=f"w3p_{rep}_{r}")
                    nc.tensor.matmul(pl[:], w3_sb[:], hidT[:, ts(r, 512)],
                                     start=True, stop=True)
                    nc.scalar.activation(logT[:, ts(r, 512)], pl[:], Identity,
                                         bias=b3_sb[:, 0:1])

                slabs = [None] * NSLAB
                for g in range(NSLAB):
                    if rep == 0 and g < len(early_slabs):
                        slabs[g] = early_slabs[g]
                    else:
                        slabs[g] = adjp.tile([128, GRP, ROWS], FP8,
                                             tag="adjT",
                                             name=f"adjT_{rep}_{g}")
                        nc.sync.dma_start(
                            slabs[g][:],
                            adjT[:, g * GRP : (g + 1) * GRP, :])
                    # produce the g chunks this slab needs; the previous
                    # slab's range-1 aggregation runs between the encoder
                    # matmuls and the G stage so tanh+copy latency is hidden
                    enc_mm(2 * g)
                    enc_mm(2 * g + 1)
                    if g >= 1:
                        agg_pairs(1, g - 1, slabs[g - 1])
                    enc_g(2 * g)
                    enc_g(2 * g + 1)
                    agg_pairs(0, g, slabs[g])
                agg_pairs(1, NSLAB - 1, slabs[NSLAB - 1])
                enc_ctx.close()
                pp_mlp = rctx.enter_context(
                    tc.tile_pool(name=f"pp_mlp_{rep}", bufs=1, space="PSUM"))
                epilogue(0)
                epilogue(1)
            nc.sync.dma_start(logitsT[:], logT[:])

    nc.compile()
    return nc


def _factor_w2m(W2):
    """Rank-127 factorization of W2m with bf16-aware least-squares refit.

    W2m.T ~= Q @ P with P = top-127 right-singular rows. On device the
    aggregate's psum row 0 carries deg (ones feature), rows 1..127 carry
    P@msg, so qt is [128,128] with a zero row 0.
    """
    W2m = W2[HID:, :]                      # [128 f, 128 hid]
    U, S, Vt = np.linalg.svd(W2m.T.astype(np.float64))  # W2m.T = U S Vt
    P = Vt[:GDIM]                          # [127, 128]
    P_bf = P.astype(BF16_NP).astype(np.float64)
    # refit Q against the bf16-rounded P actually used on device
    Q = W2m.T.astype(np.float64) @ np.linalg.pinv(P_bf)  # [128 hid, 127]
    pt = np.ascontiguousarray(P_bf.T.astype(BF16_NP))    # [128, 127]
    qt = np.zeros((HID, HID), np.float32)
    qt[1:, :] = Q.T.astype(np.float32)                   # [128, 128]
    return pt, np.ascontiguousarray(qt)


def _prep_in_maps(inputs):
    obs = np.asarray(inputs["obs_agents"], np.float32)
    adj = np.asarray(inputs["adj"])
    W1 = np.asarray(inputs["W1"], np.float32)
    b1 = np.asarray(inputs["b1"], np.float32)
    W2 = np.asarray(inputs["W2"], np.float32)
    b2 = np.asarray(inputs["b2"], np.float32)
    W3 = np.asarray(inputs["W3"], np.float32)
    b3 = np.asarray(inputs["b3"], np.float32)

    obsT = np.ascontiguousarray(obs.T)                       # [64, 8192]
    pt, qt = _factor_w2m(W2)
    w1c = np.ascontiguousarray(W1).astype(BF16_NP)
    b1c = np.ascontiguousarray(b1.reshape(HID, 1))
    w2h = np.ascontiguousarray(W2[:HID]).astype(BF16_NP)     # [128, 128]
    b2c = np.ascontiguousarray(b2.reshape(HID, 1))
    w3c = np.ascontiguousarray(W3)
    b3c = np.ascontiguousarray(b3.reshape(ACT_DIM, 1))

    adj_u8 = adj.astype(np.uint8) * np.uint8(FP8_ONE)

    in_maps = []
    for c in range(CORES):
        r0 = c * ROWS
        blk = adj_u8[r0 : r0 + ROWS]                 # [1024, 8192]
        blk = np.roll(blk, -r0, axis=1)              # own cols at j=0..1023
        adjTc = np.ascontiguousarray(
            blk.T.reshape(JCH, 128, ROWS).transpose(1, 0, 2)).view(FP8_NP)
        obsTc = np.ascontiguousarray(
            np.roll(obsT, -r0, axis=1)).astype(BF16_NP)
        in_maps.append({
            "adjT": adjTc, "obsT": obsTc, "w1": w1c, "b1": b1c,
            "pt": pt, "w2h": w2h, "qt": qt, "b2": b2c,
            "w3": w3c, "b3": b3c,
        })
    return in_maps


_CACHE = {}


def _get_exec(reps=1):
    key = ("exec", reps)
    if key in _CACHE:
        return _CACHE[key]

    import jax
    from concourse import bass2jax

    bass2jax.install_neuronx_cc_hook()
    nc = _build_nc(reps)

    partition_name = (nc.partition_id_tensor.name
                      if nc.partition_id_tensor else None)
    in_names, out_names, out_avals, out_shapes = [], [], [], []
    for alloc in nc.m.functions[0].allocations:
        if not isinstance(alloc, mybir.MemoryLocationSet):
            continue
        name = alloc.memorylocations[0].name
        if alloc.kind == "ExternalInput":
            if name != partition_name:
                in_names.append(name)
        elif alloc.kind == "ExternalOutput":
            out_names.append(name)
            shape = tuple(alloc.tensor_shape)
            dtype = mybir.dt.np(alloc.dtype)
            out_avals.append(jax.core.ShapedArray(shape, dtype))
            out_shapes.append((shape, dtype))
    n_params = len(in_names)
    all_names = tuple(in_names) + tuple(out_names)
    if partition_name is not None:
        all_names = all_names + (partition_name,)

    def _step(ins, zeros):
        extra = ((bass2jax.partition_id_tensor(),)
                 if partition_name is not None else ())
        outs = bass2jax._bass_exec_p.bind(
            *ins, *zeros, *extra,
            out_avals=tuple(out_avals),
            in_names=all_names,
            out_names=tuple(out_names),
            lowering_input_output_aliases=(),
            sim_require_finite=True,
            sim_require_nnan=True,
            nc=nc,
        )
        return tuple(outs)

    devices = jax.devices()[:CORES]
    mesh = bass2jax.Mesh(np.asarray(devices), ("core",))
    spec = bass2jax.PartitionSpec("core")
    n_outs = len(out_names)
    in_specs = (spec,) * (n_params + n_outs)
    out_specs = (spec,) * n_outs if n_outs > 1 else spec

    def _body(*args):
        outs = _step(args[:n_params], args[n_params:])
        return outs if n_outs > 1 else outs[0]

    fn = jax.jit(bass2jax.shard_map(
        _body, mesh=mesh, in_specs=in_specs, out_specs=out_specs,
        check_rep=False))

    _CACHE[key] = dict(nc=nc, fn=fn, mesh=mesh,
                       spec=spec, in_names=in_names, out_names=out_names,
                       out_shapes=out_shapes, n_params=n_params)
    return _CACHE[key]


def _concat_args(ex, in_maps):
    concat_in = [
        np.concatenate([in_maps[c][nm] for c in range(CORES)], axis=0)
        for nm in ex["in_names"]
    ]
    concat_zeros = [
        np.zeros((CORES * shape[0], *shape[1:]), dtype)
        for shape, dtype in ex["out_shapes"]
    ]
    return concat_in, concat_zeros


def _unshard_logits(ex, out_arr):
    lt = np.asarray(out_arr).reshape(CORES, ACT_DIM, ROWS)
    out = np.empty((N_AGENTS, ACT_DIM), np.float32)
    for c in range(CORES):
        out[c * ROWS : (c + 1) * ROWS] = lt[c].T
    return out


def run(inputs):
    in_maps = _prep_in_maps(inputs)
    try:
        ex = _get_exec()
        concat_in, concat_zeros = _concat_args(ex, in_maps)
        out_arr = ex["fn"](*concat_in, *concat_zeros)
        return _unshard_logits(ex, out_arr)
    except Exception:
        from concourse.bass_utils import run_bass_kernel_spmd
        if "nc" not in _CACHE:
            _CACHE["nc"] = _build_nc()
        res = run_bass_kernel_spmd(_CACHE["nc"], in_maps, list(range(CORES)))
        out = np.empty((N_AGENTS, ACT_DIM), np.float32)
        for c in range(CORES):
            out[c * ROWS : (c + 1) * ROWS] = res.results[c]["logitsT"].T
        return out


def timed_run(inputs, reps=64, calls=30):
    """Marginal per-rep device time via synchronous-call slope.

    Builds two programs (reps=1 and reps=`reps`) whose rep bodies each do the
    FULL computation (adjacency re-DMA'd from HBM, encode, aggregate, MLP,
    output store). Interleaved synchronous calls give tight wall-time
    distributions; the median difference divided by (reps-1) isolates the
    steady-state per-invocation device time, cancelling the relay RTT.
    Returns (output, per_rep_ns).
    """
    import jax
    import time

    in_maps = _prep_in_maps(inputs)
    progs = {}
    out1 = None
    ex1 = None
    for r in (1, reps):
        ex = _get_exec(reps=r)
        concat_in, concat_zeros = _concat_args(ex, in_maps)
        sharding = jax.sharding.NamedSharding(ex["mesh"], ex["spec"])
        dev_in = [jax.device_put(a, sharding) for a in concat_in]
        dev_zeros = [jax.device_put(z, sharding) for z in concat_zeros]
        out = jax.block_until_ready(ex["fn"](*dev_in, *dev_zeros))
        progs[r] = (ex["fn"], dev_in, dev_zeros)
        if r == 1:
            out1, ex1 = out, ex

    samples = {r: [] for r in progs}
    for _ in range(calls):
        for r in (1, reps):
            fn, dev_in, dev_zeros = progs[r]
            t0 = time.perf_counter()
            jax.block_until_ready(fn(*dev_in, *dev_zeros))
            samples[r].append((time.perf_counter() - t0) * 1e9)
    t1 = float(np.median(samples[1]))
    tR = float(np.median(samples[reps]))
    per_rep_ns = (tR - t1) / (reps - 1)
    print(f"sync-slope medians: r1 {t1/1e3:.0f}us  r{reps} {tR/1e3:.0f}us")
    return _unshard_logits(ex1, out1), per_rep_ns


def kernel(**inputs) -> np.ndarray:
    return run(inputs)


# revision 5
# speedup vs baseline: 1.9621x; 1.9621x over previous
"""CommNet actor kernel for Trainium2, SPMD across 8 NeuronCores.

Math (reference):
    h      = tanh(obs @ W1 + b1)                       [N, 128]
    deg    = adj.sum(1);  msg = (adj @ h) / max(deg,1) [N, 128]
    hid    = tanh(concat(h, msg) @ W2 + b2)            [N, 128]
    logits = hid @ W3 + b3                             [N, 16]

Sharding: rows (agents) of adj split across 8 cores, 1024 rows each; no
collectives (each core recomputes the full h, which is cheap).

Key idea (vs the naive two-stream aggregation): msg enters the MLP only
through W2m (the second half of W2), and aggregation commutes with linear
projection: W2m.T @ (adj@h).T = Q @ (adj @ (h@P.T)).T for any factorization
W2m.T = Q P. Using the rank-127 SVD of W2m (dropping sigma_128 costs ~3e-4
relative error), P is [127,128], so the aggregated feature vector
g = [1, P h] fits the 128 stationary columns WITH a ones column: the same
fp8 DoubleRow stream that aggregates messages also produces deg in psum
row 0 for free. The second full adjacency stream for deg (~14us of PE
time) disappears, and the uniform g-chunk stationary sequence lets the PE
pipeline weight loads (the alternating h/ones stationaries thrashed them).

Layouts: h is computed feature-major (hT = tanh(W1.T obsT + b1), bias via
the ACT per-partition bias) and serves both the G projection stage and the
MLP's exact-h path. A G-stage (64 matmuls, stationary = hT chunk, moving =
P.T) produces agent-major fp8 g chunks for the aggregation stationary.
Inputs are rotated per-core so own rows sit at agent positions 0..1023
(the SPMD program is identical across cores; only data differs).

Scheduling: per adjacency slab, the two encoder matmul windows are emitted
first, then the previous slab's range-1 aggregation (covers the tanh +
psum-to-fp8 copy latency on ACT/DVE), then the G matmuls and the current
slab's range-0 aggregation. Stage tiles are double-buffered so rep r+1's
encoder overlaps rep r's epilogue.

adj is cast host-side to fp8 (0/1 exact) and pre-transposed/tiled so DMAs
are large and contiguous: 8.4 MB of adjacency per core per invocation.
"""

import numpy as np
import ml_dtypes
from contextlib import ExitStack

import concourse.tile as tile
from concourse import bacc, mybir
from concourse.bass import ts

N_AGENTS, OBS_DIM, HID, ACT_DIM = 8192, 64, 128, 16
CORES = 8
ROWS = N_AGENTS // CORES          # 1024 rows per core
JCH = N_AGENTS // 128             # 64 contraction chunks
GRP = 8                           # j-chunks per adjacency DMA slab
GDIM = HID - 1                    # 127 projected features (+1 ones)

F32 = mybir.dt.float32
F32R = mybir.dt.float32r
BF16 = mybir.dt.bfloat16
FP8 = mybir.dt.float8e4
BF16_NP = ml_dtypes.bfloat16
FP8_NP = ml_dtypes.float8_e4m3
FP8_ONE = 0x38  # bit pattern of 1.0 in e4m3

Tanh = mybir.ActivationFunctionType.Tanh
Identity = mybir.ActivationFunctionType.Identity


def _build_nc(reps=1):
    nc = bacc.Bacc("TRN2", target_bir_lowering=False, debug=False,
                   num_devices=CORES)

    adjT = nc.dram_tensor("adjT", [128, JCH, ROWS], FP8, kind="ExternalInput")
    obsT = nc.dram_tensor("obsT", [OBS_DIM, N_AGENTS], BF16,
                          kind="ExternalInput")
    w1 = nc.dram_tensor("w1", [OBS_DIM, HID], BF16, kind="ExternalInput")
    b1 = nc.dram_tensor("b1", [HID, 1], F32, kind="ExternalInput")
    pt = nc.dram_tensor("pt", [HID, GDIM], BF16, kind="ExternalInput")
    w2h = nc.dram_tensor("w2h", [HID, HID], BF16, kind="ExternalInput")
    # qt row 0 is zero: it multiplies the deg row of the aggregate
    qt = nc.dram_tensor("qt", [HID, HID], F32R, kind="ExternalInput")
    b2 = nc.dram_tensor("b2", [HID, 1], F32, kind="ExternalInput")
    w3 = nc.dram_tensor("w3", [HID, ACT_DIM], F32R, kind="ExternalInput")
    b3 = nc.dram_tensor("b3", [ACT_DIM, 1], F32, kind="ExternalInput")
    logitsT = nc.dram_tensor("logitsT", [ACT_DIM, ROWS], F32,
                             kind="ExternalOutput")

    DR = mybir.MatmulPerfMode.DoubleRow
    NR = ROWS // 512        # 2 output ranges per core
    NSLAB = JCH // GRP      # 8 adjacency slabs
    NW = 16                 # agent windows of 512 (4 chunks each)
    with tile.TileContext(nc) as tc, ExitStack() as ctx:
        consts = ctx.enter_context(tc.tile_pool(name="consts", bufs=1))
        stage = ctx.enter_context(tc.tile_pool(name="stage", bufs=1))
        stage2 = ctx.enter_context(tc.tile_pool(name="stage2", bufs=2))
        adjp = ctx.enter_context(tc.tile_pool(name="adjp", bufs=NSLAB))

        w1_sb = consts.tile([OBS_DIM, HID], BF16, tag="w1")
        nc.sync.dma_start(w1_sb[:], w1[:])
        b1_sb = consts.tile([HID, 1], F32, tag="b1")
        nc.sync.dma_start(b1_sb[:], b1[:])
        pt_sb = consts.tile([HID, GDIM], BF16, tag="pt")
        nc.sync.dma_start(pt_sb[:], pt[:])
        # obsT split into 4 tiles so E1' can start early
        OCH = 4
        ow = N_AGENTS // OCH
        obsT_sbs = []
        early_slabs = []
        for oc in range(OCH):
            t = consts.tile([OBS_DIM, ow], BF16, tag=f"obsT{oc}",
                            name=f"obsT{oc}")
            nc.sync.dma_start(t[:], obsT[:, oc * ow : (oc + 1) * ow])
            obsT_sbs.append(t)
            if oc < 2:
                es = adjp.tile([128, GRP, ROWS], FP8, tag="adjT",
                               name=f"adjT_0_{oc}")
                nc.sync.dma_start(es[:], adjT[:, oc * GRP : (oc + 1) * GRP, :])
                early_slabs.append(es)
        w2h_sb = consts.tile([HID, HID], BF16, tag="w2h")
        nc.sync.dma_start(w2h_sb[:], w2h[:])
        qt_sb = consts.tile([HID, HID], F32R, tag="qt")
        nc.sync.dma_start(qt_sb[:], qt[:])
        b2_sb = consts.tile([HID, 1], F32, tag="b2")
        nc.sync.dma_start(b2_sb[:], b2[:])
        w3_sb = consts.tile([HID, ACT_DIM], F32R, tag="w3")
        nc.sync.dma_start(w3_sb[:], w3[:])
        b3_sb = consts.tile([ACT_DIM, 1], F32, tag="b3")
        nc.sync.dma_start(b3_sb[:], b3[:])
        ones_bc = consts.tile([1, 128], F32R, tag="ones_bc")
        nc.vector.memset(ones_bc[:].bitcast(mybir.dt.uint32), 0x3F800000)

        def alloc_hg(rep):
            hT = stage2.tile([HID, N_AGENTS], BF16, tag="hT",
                             name=f"hT_{rep}")
            g_sb = stage2.tile([128, JCH, HID], FP8, tag="g_sb",
                               name=f"g_sb_{rep}")
            # ones feature (column 0 of every chunk) -> deg in psum row 0
            nc.vector.memset(
                g_sb[:, :, 0:1].bitcast(mybir.dt.uint8), FP8_ONE)
            return hT, g_sb

        carry = None
        for rep in range(reps):
            carried = carry is not None
            hT, g_sb = carry if carried else alloc_hg(rep)
            msgT = stage2.tile([HID, ROWS], F32R, tag="msgT",
                               name=f"msgT_{rep}")
            hidT = stage2.tile([HID, ROWS], F32R, tag="hidT",
                               name=f"hidT_{rep}")
            logT = stage2.tile([ACT_DIM, ROWS], F32, tag="logT",
                               name=f"logT_{rep}")

            with ExitStack() as rctx:
                pp_agg = rctx.enter_context(
                    tc.tile_pool(name=f"pp_agg_{rep}", bufs=1, space="PSUM"))
                msgps = [pp_agg.tile([128, 512], F32, tag=f"msgps{r}",
                                     name=f"msgps_{rep}_{r}")
                         for r in range(NR)]
                enc_ctx = ExitStack()
                pp_enc = enc_ctx.enter_context(
                    tc.tile_pool(name=f"pp_enc_{rep}", bufs=2, space="PSUM"))

                def enc_mm(q, tgt_h=None, pool=None, tag="e1", rr=None):
                    # E1': hT[:, win] = tanh(W1.T @ obsT[:, win] + b1)
                    tgt_h = hT if tgt_h is None else tgt_h
                    pool = pp_enc if pool is None else pool
                    rr = rep if rr is None else rr
                    osb = obsT_sbs[q * 512 // ow]
                    ocol = (q * 512) % ow
                    ps = pool.tile([HID, 512], F32, tag=tag,
                                   name=f"e1_{rr}_{q}")
                    nc.tensor.matmul(ps[:], w1_sb[:],
                                     osb[:, ocol : ocol + 512],
                                     start=True, stop=True)
                    nc.scalar.activation(tgt_h[:, ts(q, 512)], ps[:], Tanh,
                                         bias=b1_sb[:, 0:1])

                def enc_g(q, tgt_h=None, tgt_g=None, pool=None, tag="g",
                          rr=None):
                    # G: agent-major projected features, 4 chunks per bank
                    tgt_h = hT if tgt_h is None else tgt_h
                    tgt_g = g_sb if tgt_g is None else tgt_g
                    pool = pp_enc if pool is None else pool
                    rr = rep if rr is None else rr
                    gp = pool.tile([128, 4, GDIM], F32, tag=tag,
                                   name=f"g_{rr}_{q}")
                    for k in range(4):
                        c = 4 * q + k
                        nc.tensor.matmul(gp[:, k, :],
                                         tgt_h[:, c * 128 : (c + 1) * 128],
                                         pt_sb[:], start=True, stop=True)
                    nc.vector.tensor_copy(
                        tgt_g[:, 4 * q : 4 * q + 4, 1:HID], gp[:])

                def agg_pairs(r, g, slab):
                    for jj2 in range(GRP // 2):
                        j = g * GRP + 2 * jj2
                        first = (g == 0 and jj2 == 0)
                        last = (g == NSLAB - 1 and jj2 == GRP // 2 - 1)
                        nc.tensor.matmul(msgps[r][:],
                                         g_sb[:, j : j + 2, :],
                                         slab[:, 2 * jj2 : 2 * jj2 + 2,
                                              ts(r, 512)],
                                         start=first, stop=last,
                                         perf_mode=DR)

                def epilogue(r):
                    # deg is row 127 of the aggregate
                    dmax = stage.tile([1, 512], F32R, tag="dmax",
                                      name=f"dmax_{rep}_{r}")
                    nc.vector.tensor_scalar_max(
                        dmax[:], msgps[r][0:1, :], 1.0)
                    bc = pp_mlp.tile([128, 512], F32, tag="bc",
                                     name=f"bc_{rep}_{r}")
                    nc.tensor.matmul(bc[:], ones_bc[:], dmax[:],
                                     start=True, stop=True)
                    recip = stage.tile([128, 512], F32, tag="recip",
                                       name=f"recip_{rep}_{r}")
                    nc.vector.reciprocal(recip[:], bc[:])
                    nc.vector.tensor_tensor(msgT[:, ts(r, 512)],
                                            msgps[r][:], recip[:],
                                            mybir.AluOpType.mult)
                    pw = pp_mlp.tile([128, 512], F32, tag="w2p", bufs=2,
                                     name=f"w2p_{rep}_{r}")
                    nc.tensor.matmul(pw[:], w2h_sb[:], hT[:, ts(r, 512)],
                                     start=True, stop=False)
                    nc.tensor.matmul(pw[:], qt_sb[:],
                                     msgT[:, ts(r, 512)],
                                     start=False, stop=True)
                    nc.scalar.activation(hidT[:, ts(r, 512)], pw[:], Tanh,
                                         bias=b2_sb[:, 0:1])
                    pl = pp_mlp.tile([ACT_DIM, 512], F32, tag="w3p",
                                     name=f"w3p_{rep}_{r}")
                    nc.tensor.matmul(pl[:], w3_sb[:], hidT[:, ts(r, 512)],
                                     start=True, stop=True)
                    nc.scalar.activation(logT[:, ts(r, 512)], pl[:], Identity,
                                         bias=b3_sb[:, 0:1])

                slabs = [None] * NSLAB
                for g in range(NSLAB):
                    if rep == 0 and g < len(early_slabs):
                        slabs[g] = early_slabs[g]
                    else:
                        slabs[g] = adjp.tile([128, GRP, ROWS], FP8,
                                             tag="adjT",
                                             name=f"adjT_{rep}_{g}")
                        nc.sync.dma_start(
                            slabs[g][:],
                            adjT[:, g * GRP : (g + 1) * GRP, :])
                    # produce the g chunks this slab needs; the previous
                    # slab's range-1 aggregation runs between the encoder
                    # matmuls and the G stage so tanh+copy latency is hidden
                    enc_mm(2 * g)
                    enc_mm(2 * g + 1)
                    if g >= 1:
                        agg_pairs(1, g - 1, slabs[g - 1])
                    enc_g(2 * g)
                    enc_g(2 * g + 1)
                    agg_pairs(0, g, slabs[g])
                agg_pairs(1, NSLAB - 1, slabs[NSLAB - 1])
                enc_ctx.close()
                pp_mlp = rctx.enter_context(
                    tc.tile_pool(name=f"pp_mlp_{rep}", bufs=1, space="PSUM"))
                epilogue(0)
                epilogue(1)
            nc.sync.dma_start(logitsT[:], logT[:])

    nc.compile()
    return nc


def _factor_w2m(W2):
    """Rank-127 factorization of W2m with bf16-aware least-squares refit.

    W2m.T ~= Q @ P with P = top-127 right-singular rows. On device the
    aggregate's psum row 0 carries deg (ones feature), rows 1..127 carry
    P@msg, so qt is [128,128] with a zero row 0.
    """
    W2m = W2[HID:, :]                      # [128 f, 128 hid]
    U, S, Vt = np.linalg.svd(W2m.T.astype(np.float64))  # W2m.T = U S Vt
    P = Vt[:GDIM]                          # [127, 128]
    P_bf = P.astype(BF16_NP).astype(np.float64)
    # refit Q against the bf16-rounded P actually used on device
    Q = W2m.T.astype(np.float64) @ np.linalg.pinv(P_bf)  # [128 hid, 127]
    pt = np.ascontiguousarray(P_bf.T.astype(BF16_NP))    # [128, 127]
    qt = np.zeros((HID, HID), np.float32)
    qt[1:, :] = Q.T.astype(np.float32)                   # [128, 128]
    return pt, np.ascontiguousarray(qt)


def _prep_in_maps(inputs):
    obs = np.asarray(inputs["obs_agents"], np.float32)
    adj = np.asarray(inputs["adj"])
    W1 = np.asarray(inputs["W1"], np.float32)
    b1 = np.asarray(inputs["b1"], np.float32)
    W2 = np.asarray(inputs["W2"], np.float32)
    b2 = np.asarray(inputs["b2"], np.float32)
    W3 = np.asarray(inputs["W3"], np.float32)
    b3 = np.asarray(inputs["b3"], np.float32)

    obsT = np.ascontiguousarray(obs.T)                       # [64, 8192]
    pt, qt = _factor_w2m(W2)
    w1c = np.ascontiguousarray(W1).astype(BF16_NP)
    b1c = np.ascontiguousarray(b1.reshape(HID, 1))
    w2h = np.ascontiguousarray(W2[:HID]).astype(BF16_NP)     # [128, 128]
    b2c = np.ascontiguousarray(b2.reshape(HID, 1))
    w3c = np.ascontiguousarray(W3)
    b3c = np.ascontiguousarray(b3.reshape(ACT_DIM, 1))

    adj_u8 = adj.astype(np.uint8) * np.uint8(FP8_ONE)

    in_maps = []
    for c in range(CORES):
        r0 = c * ROWS
        blk = adj_u8[r0 : r0 + ROWS]                 # [1024, 8192]
        blk = np.roll(blk, -r0, axis=1)              # own cols at j=0..1023
        adjTc = np.ascontiguousarray(
            blk.T.reshape(JCH, 128, ROWS).transpose(1, 0, 2)).view(FP8_NP)
        obsTc = np.ascontiguousarray(
            np.roll(obsT, -r0, axis=1)).astype(BF16_NP)
        in_maps.append({
            "adjT": adjTc, "obsT": obsTc, "w1": w1c, "b1": b1c,
            "pt": pt, "w2h": w2h, "qt": qt, "b2": b2c,
            "w3": w3c, "b3": b3c,
        })
    return in_maps


_CACHE = {}


def _get_exec(reps=1):
    key = ("exec", reps)
    if key in _CACHE:
        return _CACHE[key]

    import jax
    from concourse import bass2jax

    bass2jax.install_neuronx_cc_hook()
    nc = _build_nc(reps)

    partition_name = (nc.partition_id_tensor.name
                      if nc.partition_id_tensor else None)
    in_names, out_names, out_avals, out_shapes = [], [], [], []
    for alloc in nc.m.functions[0].allocations:
        if not isinstance(alloc, mybir.MemoryLocationSet):
            continue
        name = alloc.memorylocations[0].name
        if alloc.kind == "ExternalInput":
            if name != partition_name:
                in_names.append(name)
        elif alloc.kind == "ExternalOutput":
            out_names.append(name)
            shape = tuple(alloc.tensor_shape)
            dtype = mybir.dt.np(alloc.dtype)
            out_avals.append(jax.core.ShapedArray(shape, dtype))
            out_shapes.append((shape, dtype))
    n_params = len(in_names)
    all_names = tuple(in_names) + tuple(out_names)
    if partition_name is not None:
        all_names = all_names + (partition_name,)

    def _step(ins, zeros):
        extra = ((bass2jax.partition_id_tensor(),)
                 if partition_name is not None else ())
        outs = bass2jax._bass_exec_p.bind(
            *ins, *zeros, *extra,
            out_avals=tuple(out_avals),
            in_names=all_names,
            out_names=tuple(out_names),
            lowering_input_output_aliases=(),
            sim_require_finite=True,
            sim_require_nnan=True,
            nc=nc,
        )
        return tuple(outs)

    devices = jax.devices()[:CORES]
    mesh = bass2jax.Mesh(np.asarray(devices), ("core",))
    spec = bass2jax.PartitionSpec("core")
    n_outs = len(out_names)
    in_specs = (spec,) * (n_params + n_outs)
    out_specs = (spec,) * n_outs if n_outs > 1 else spec

    def _body(*args):
        outs = _step(args[:n_params], args[n_params:])
        return outs if n_outs > 1 else outs[0]

    fn = jax.jit(bass2jax.shard_map(
        _body, mesh=mesh, in_specs=in_specs, out_specs=out_specs,
        check_rep=False))

    _CACHE[key] = dict(nc=nc, fn=fn, mesh=mesh,
                       spec=spec, in_names=in_names, out_names=out_names,
                       out_shapes=out_shapes, n_params=n_params)
    return _CACHE[key]


def _concat_args(ex, in_maps):
    concat_in = [
        np.concatenate([in_maps[c][nm] for c in range(CORES)], axis=0)
        for nm in ex["in_names"]
    ]
    concat_zeros = [
        np.zeros((CORES * shape[0], *shape[1:]), dtype)
        for shape, dtype in ex["out_shapes"]
    ]
    return concat_in, concat_zeros


def _unshard_logits(ex, out_arr):
    lt = np.asarray(out_arr).reshape(CORES, ACT_DIM, ROWS)
    out = np.empty((N_AGENTS, ACT_DIM), np.float32)
    for c in range(CORES):
        out[c * ROWS : (c + 1) * ROWS] = lt[c].T
    return out


def run(inputs):
    in_maps = _prep_in_maps(inputs)
    try:
        ex = _get_exec()
        concat_in, concat_zeros = _concat_args(ex, in_maps)
        out_arr = ex["fn"](*concat_in, *concat_zeros)
        return _unshard_logits(ex, out_arr)
    except Exception:
        from concourse.bass_utils import run_bass_kernel_spmd
        if "nc" not in _CACHE:
            _CACHE["nc"] = _build_nc()
        res = run_bass_kernel_spmd(_CACHE["nc"], in_maps, list(range(CORES)))
        out = np.empty((N_AGENTS, ACT_DIM), np.float32)
        for c in range(CORES):
            out[c * ROWS : (c + 1) * ROWS] = res.results[c]["logitsT"].T
        return out


def timed_run(inputs, reps=64, calls=30):
    """Marginal per-rep device time via synchronous-call slope.

    Builds two programs (reps=1 and reps=`reps`) whose rep bodies each do the
    FULL computation (adjacency re-DMA'd from HBM, encode, aggregate, MLP,
    output store). Interleaved synchronous calls give tight wall-time
    distributions; the median difference divided by (reps-1) isolates the
    steady-state per-invocation device time, cancelling the relay RTT.
    Returns (output, per_rep_ns).
    """
    import jax
    import time

    in_maps = _prep_in_maps(inputs)
    progs = {}
    out1 = None
    ex1 = None
    for r in (1, reps):
        ex = _get_exec(reps=r)
        concat_in, concat_zeros = _concat_args(ex, in_maps)
        sharding = jax.sharding.NamedSharding(ex["mesh"], ex["spec"])
        dev_in = [jax.device_put(a, sharding) for a in concat_in]
        dev_zeros = [jax.device_put(z, sharding) for z in concat_zeros]
        out = jax.block_until_ready(ex["fn"](*dev_in, *dev_zeros))
        progs[r] = (ex["fn"], dev_in, dev_zeros)
        if r == 1:
            out1, ex1 = out, ex

    samples = {r: [] for r in progs}
    for _ in range(calls):
        for r in (1, reps):
            fn, dev_in, dev_zeros = progs[r]
            t0 = time.perf_counter()
            jax.block_until_ready(fn(*dev_in, *dev_zeros))
            samples[r].append((time.perf_counter() - t0) * 1e9)
    t1 = float(np.median(samples[1]))
    tR = float(np.median(samples[reps]))
    per_rep_ns = (tR - t1) / (reps - 1)
    print(f"sync-slope medians: r1 {t1/1e3:.0f}us  r{reps} {tR/1e3:.0f}us")
    return _unshard_logits(ex1, out1), per_rep_ns


def kernel(**inputs) -> np.ndarray:
    return run(inputs)
